# revision 3
# baseline (speedup 1.0000x reference)
"""Chamfer distance kernel for Trainium2 (8 NeuronCores, data-parallel over batch).

Input : x, y float32 [16, 4096, 3]
Output: scalar float32 = mean_b [ mean_n min_m ||x_bn - y_bm||^2
                                + mean_m min_n ||x_bn - y_bm||^2 ]

Strategy (per core, 2 batches):
  u[n,m] = 2 x_n.y_m - |x_n|^2 - |y_m|^2  = -||x_n - y_m||^2
  computed tile-wise on the PE as a single K=13 bf16 matmul using hi/lo
  splits of the operands (fp32-grade accuracy at bf16 speed, ~2e-4 absmax):
    l = (2xh, 2xl, 2xh, x2h, x2l, 1, 1)[13, n]   r = (yh, yh, yl, -1, -1, -y2h, -y2l)[13, m]
  Per [128, 2048] PSUM tile: ScalarE casts to fp16 SBUF; VectorE then
  - dirA (min over m): pairwise TT-max tree 2048->256 + reduce_max -> row max
  - dirB (min over n): running elementwise TT-max into per-column buffers,
    finalized by PE transposes + grouped reduce_max.
  Means via reduce_sum + ones-matmul; batch results accumulate in PSUM.
Host: builds augmented operands (O(B*N)), shards batches 2-per-core, sums the
8 partial sums and divides by 16.
"""
import sys

sys.path.insert(0, "/opt/trn_rl_repo")

import numpy as np

import concourse.bacc as bacc
import concourse.tile as tile
from concourse import mybir
from concourse.alu_op_type import AluOpType
from concourse.bass_utils import run_bass_kernel_spmd

F32 = mybir.dt.float32
BF16 = mybir.dt.bfloat16
F16 = mybir.dt.float16
MAXOP = AluOpType.max
X = mybir.AxisListType.X

B, N, D3 = 16, 4096, 3
NCORES = 8
BPC = B // NCORES           # batches per core
RB = N // 128               # 32 row blocks
G = 2                       # two 2048-wide column groups
GW = 2048


def _build_nc(repeat: int = 1):
    nc = bacc.Bacc("TRN2", target_bir_lowering=False, debug=False, num_devices=NCORES)
    xl_d = nc.dram_tensor("xl", [13, BPC * N], BF16, kind="ExternalInput").ap()
    yr_d = nc.dram_tensor("yr", [13, BPC * N], BF16, kind="ExternalInput").ap()
    id_d = nc.dram_tensor("ident", [128, 128], F16, kind="ExternalInput").ap()
    out_d = nc.dram_tensor("out", [1, 1], F32, kind="ExternalOutput").ap()

    with tile.TileContext(nc) as tc:
        import contextlib
        with contextlib.ExitStack() as ctx:
            const = ctx.enter_context(tc.tile_pool(name="const", bufs=1))
            acc = ctx.enter_context(tc.tile_pool(name="acc", bufs=1))
            uhp = ctx.enter_context(tc.tile_pool(name="uhp", bufs=6))
            tree = ctx.enter_context(tc.tile_pool(name="tree", bufs=2))
            pmm = ctx.enter_context(tc.tile_pool(name="pmm", bufs=2, space="PSUM"))

            xl_t = const.tile([13, BPC * N], BF16, name="xl_t")
            nc.gpsimd.dma_start(xl_t[:], xl_d[:])
            yr_t = const.tile([13, BPC * N], BF16, name="yr_t")
            nc.gpsimd.dma_start(yr_t[:], yr_d[:])
            id_t = const.tile([128, 128], F16, name="id_t")
            nc.gpsimd.dma_start(id_t[:], id_d[:])
            ones_t = const.tile([128, 1], F32, name="ones_t")
            nc.vector.memset(ones_t[:], 1.0)

            colrun = [acc.tile([128, GW], F16, name=f"colrun{g}") for g in range(G)]
            rowacc = acc.tile([128, RB], F32, name="rowacc")
            colB = acc.tile([128, RB], F32, name="colB")
            svs = acc.tile([128, 1], F32, name="svs")
            sv2 = acc.tile([128, 1], F32, name="sv2")
            s_out = acc.tile([1, 1], F32, name="s_out")

            nmm = 0
            for b in range(BPC * repeat):
                boff = (b % BPC) * N
                for r in range(RB):
                    lhs = xl_t[:, boff + 128 * r : boff + 128 * (r + 1)]
                    uh = []
                    for g in range(G):
                        P = pmm.tile([128, GW], F32, name="P", tag="mm")
                        for c in range(4):
                            moff = boff + GW * g + 512 * c
                            nc.tensor.matmul(
                                P[:, 512 * c : 512 * (c + 1)],
                                lhs, yr_t[:, moff : moff + 512],
                                start=True, stop=True)
                            nmm += 1
                        u = uhp.tile([128, GW], F16, name="uh", tag="uh")
                        nc.scalar.copy(u[:], P[:])
                        uh.append(u)
                        # dirB running column max
                        if r == 0:
                            nc.vector.tensor_copy(colrun[g][:], u[:])
                        else:
                            nc.vector.tensor_max(colrun[g][:], colrun[g][:], u[:])
                    # dirA tree: 2x2048 -> 256, then reduce
                    a1 = tree.tile([128, 2048], F16, name="a1")
                    nc.vector.tensor_max(a1[:], uh[0][:], uh[1][:])
                    a2 = tree.tile([128, 1024], F16, name="a2")
                    nc.vector.tensor_max(a2[:], a1[:, 0:1024], a1[:, 1024:2048])
                    a3 = tree.tile([128, 512], F16, name="a3")
                    nc.vector.tensor_max(a3[:], a2[:, 0:512], a2[:, 512:1024])
                    a4 = tree.tile([128, 256], F16, name="a4")
                    nc.vector.tensor_max(a4[:], a3[:, 0:256], a3[:, 256:512])
                    nc.vector.reduce_max(rowacc[:, r : r + 1], a4[:], axis=X)

                # batch finalize: dirB cross-partition max via PE transposes
                for g in range(G):
                    ptp = pmm.tile([128, GW], F16, name="ptp", tag="mm")
                    for j in range(16):
                        nc.tensor.matmul(
                            ptp[:, 128 * j : 128 * (j + 1)],
                            colrun[g][:, 128 * j : 128 * (j + 1)], id_t[:],
                            is_transpose=True, start=(j % 8 == 0), stop=(j % 8 == 7),
                            skip_group_check=True)
                    nc.vector.reduce_max(
                        colB[:, 16 * g : 16 * (g + 1)],
                        ptp[:].rearrange("p (j f) -> p j f", j=16), axis=X)

                # per-batch scalar contribution accumulated in SBUF [128,1]
                sv = acc.tile([128, 1], F32, name=f"sv_{b}")
                nc.vector.reduce_sum(sv[:], rowacc[:], axis=X)
                nc.vector.reduce_sum(sv2[:], colB[:], axis=X)
                nc.vector.tensor_add(sv[:], sv[:], sv2[:])
                if b == 0:
                    nc.vector.tensor_copy(svs[:], sv[:])
                else:
                    nc.vector.tensor_add(svs[:], svs[:], sv[:])

            p_s = pmm.tile([1, 1], F32, name="p_s", tag="mm")
            nc.tensor.matmul(p_s[:], svs[:], ones_t[:], start=True, stop=True)
            nc.scalar.mul(s_out[:], p_s[:], -1.0 / (N * repeat))
            nc.gpsimd.dma_start(out_d[:], s_out[:])
    nc.compile()
    return nc


def _bf16_round(a):
    a = np.ascontiguousarray(a, np.float32)
    b = a.view(np.uint32)
    return (((b + 0x7FFF + ((b >> 16) & 1)) & 0xFFFF0000).astype(np.uint32)).view(np.float32)


def _build_operands(x, y):
    """x,y [B,N,3] f32 -> per-core xl [13, BPC*N] bf16 (lhsT aug of x),
    yr [13, BPC*N] bf16 (rhs aug of y)."""
    import ml_dtypes
    bf16 = ml_dtypes.bfloat16

    def aug(px, py):
        # px: points used as lhsT (rows n), py: points as rhs (cols m); [N,3]
        xh = _bf16_round(px); xxl = (px - xh).astype(np.float32)
        yh = _bf16_round(py); yyl = (py - yh).astype(np.float32)
        x2 = np.einsum("nd,nd->n", px.astype(np.float64), px.astype(np.float64)).astype(np.float32)
        y2 = np.einsum("nd,nd->n", py.astype(np.float64), py.astype(np.float64)).astype(np.float32)
        x2h = _bf16_round(x2); x2l = (x2 - x2h).astype(np.float32)
        y2h = _bf16_round(y2); y2l = (y2 - y2h).astype(np.float32)
        one = np.ones_like(x2)
        l = np.stack([2*xh[:, 0], 2*xh[:, 1], 2*xh[:, 2],
                      2*xxl[:, 0], 2*xxl[:, 1], 2*xxl[:, 2],
                      2*xh[:, 0], 2*xh[:, 1], 2*xh[:, 2],
                      x2h, x2l, one, one])
        oy = np.ones_like(y2)
        r = np.stack([yh[:, 0], yh[:, 1], yh[:, 2],
                      yh[:, 0], yh[:, 1], yh[:, 2],
                      yyl[:, 0], yyl[:, 1], yyl[:, 2],
                      -oy, -oy, -y2h, -y2l])
        return l.astype(bf16), r.astype(bf16)

    ident = np.eye(128, dtype=np.float16)
    in_maps = []
    for core in range(NCORES):
        ls, rs = [], []
        for b in range(core * BPC, (core + 1) * BPC):
            l, r = aug(x[b], y[b])
            ls.append(l)
            rs.append(r)
        in_maps.append({
            "xl": np.concatenate(ls, axis=1),
            "yr": np.concatenate(rs, axis=1),
            "ident": ident,
        })
    return in_maps


_NC_CACHE = {}


def _get_nc(repeat: int = 1):
    if repeat not in _NC_CACHE:
        _NC_CACHE[repeat] = _build_nc(repeat)
    return _NC_CACHE[repeat]


def kernel(x, y):
    x = np.asarray(x, dtype=np.float32)
    y = np.asarray(y, dtype=np.float32)
    assert x.shape == (B, N, D3) and y.shape == (B, N, D3)
    in_maps = _build_operands(x, y)
    nc = _get_nc(1)
    res = run_bass_kernel_spmd(nc, in_maps, core_ids=list(range(NCORES)))
    total = sum(float(res.results[i]["out"][0, 0]) for i in range(NCORES))
    return np.float32(total / B)


# revision 8
# speedup vs baseline: 3.1273x; 3.1273x over previous
"""Chamfer distance kernel for Trainium2 (8 NeuronCores, data-parallel over batch).

Input : x, y float32 [16, 4096, 3]
Output: scalar float32 = mean_b [ mean_n min_m ||x_bn - y_bm||^2
                                + mean_m min_n ||x_bn - y_bm||^2 ]

Per core (2 batches). For each batch and each 128-row block of x points:
  s_k = Square(-yb_k + x_k)    (ScalarE; yb_k = y coord k broadcast to all
                                partitions [128,4096], x_k per-partition bias)
  d   = s_0 + s_1 + s_2        (VectorE adds)   -> d[p, m] = ||x_n - y_m||^2
  dirA: reduce_min(d) over m   -> row NN dist    (VectorE)
  dirB: colrun = min(colrun,d) running over row blocks (VectorE)
Batch finalize: colrun -> negate -> gpsimd partition_all_reduce(max) -> per-m
NN dists; reduce_sums + partition_all_reduce(add) -> scalar; accumulate.
Host: builds coordinate layouts (O(B*N)), shards batches 2-per-core, sums 8
partial sums / 16.
"""
import sys

sys.path.insert(0, "/opt/trn_rl_repo")

import numpy as np

import concourse.bacc as bacc
import concourse.bass as bass
import concourse.bass_isa as bass_isa
import concourse.tile as tile
from concourse import mybir
from concourse.alu_op_type import AluOpType
from concourse.bass_utils import run_bass_kernel_spmd

F32 = mybir.dt.float32
X = mybir.AxisListType.X
MIN = AluOpType.min
Square = mybir.ActivationFunctionType.Square

B, N, D3 = 16, 4096, 3
NCORES = 8
BPC = B // NCORES           # batches per core
RB = N // 128               # 32 row blocks


def _build_nc(repeat: int = 1):
    nc = bacc.Bacc("TRN2", target_bir_lowering=False, debug=False, num_devices=NCORES)
    # ys[b*3+k, m] = y[b, m, k];  xs[p, b*96 + k*32 + r] = x[b, 128r+p, k]
    ys_d = nc.dram_tensor("ys", [BPC * 3, N], F32, kind="ExternalInput").ap()
    xs_d = nc.dram_tensor("xs", [128, BPC * 3 * RB], F32, kind="ExternalInput").ap()
    out_d = nc.dram_tensor("out", [1, 1], F32, kind="ExternalOutput").ap()

    with tile.TileContext(nc) as tc:
        import contextlib
        with contextlib.ExitStack() as ctx:
            const = ctx.enter_context(tc.tile_pool(name="const", bufs=1))
            acc = ctx.enter_context(tc.tile_pool(name="acc", bufs=1))
            ybp = ctx.enter_context(tc.tile_pool(name="ybp", bufs=3))
            sq = ctx.enter_context(tc.tile_pool(name="sq", bufs=3))
            dp = ctx.enter_context(tc.tile_pool(name="dp", bufs=2))

            xs_t = const.tile([128, BPC * 3 * RB], F32, name="xs_t")
            nc.gpsimd.dma_start(xs_t[:], xs_d[:])

            def load_yb(b):
                tiles = []
                for k in range(3):
                    t = ybp.tile([128, N], F32, name=f"yb{k}", tag="yb")
                    src = ys_d[3 * b + k : 3 * b + k + 1, :]
                    bcast = bass.AP(tensor=src.tensor, offset=src.offset,
                                    ap=[[0, 128]] + [list(p) for p in src.ap[1:]])
                    nc.gpsimd.dma_start(t[:], bcast)
                    tiles.append(t)
                return tiles

            colrun = acc.tile([128, N], F32, name="colrun")
            rowacc = acc.tile([128, RB], F32, name="rowacc")
            stot = acc.tile([1, 1], F32, name="stot")
            s_out = acc.tile([1, 1], F32, name="s_out")

            for it in range(BPC * repeat):
                b = it % BPC
                yb = load_yb(b)
                for r in range(RB):
                    s = []
                    for k in range(3):
                        sk = sq.tile([128, N], F32, name="sk", tag="sk")
                        nc.scalar.activation(
                            sk[:], yb[k][:], Square,
                            bias=xs_t[:, b * 96 + k * RB + r : b * 96 + k * RB + r + 1],
                            scale=-1.0)
                        s.append(sk)
                    a01 = dp.tile([128, N], F32, name="a01", tag="a01")
                    nc.vector.tensor_add(a01[:], s[0][:], s[1][:])
                    d = dp.tile([128, N], F32, name="d", tag="d")
                    nc.vector.tensor_add(d[:], a01[:], s[2][:])
                    if r == 0:
                        nc.vector.tensor_copy(colrun[:], d[:])
                    else:
                        nc.vector.tensor_tensor(colrun[:], colrun[:], d[:], op=MIN)
                    nc.vector.tensor_reduce(rowacc[:, r : r + 1], d[:], axis=X, op=MIN)

                # batch finalize
                rs = acc.tile([128, 1], F32, name=f"rs_{it}")
                nc.vector.reduce_sum(rs[:], rowacc[:], axis=X)
                rsr = acc.tile([128, 1], F32, name=f"rsr_{it}")
                nc.gpsimd.partition_all_reduce(rsr[:], rs[:], channels=128,
                                               reduce_op=bass_isa.ReduceOp.add)
                cneg = dp.tile([128, N], F32, name="cneg", tag="a01")
                nc.vector.tensor_scalar_mul(cneg[:], colrun[:], -1.0)
                cred = dp.tile([128, N], F32, name="cred", tag="d")
                nc.gpsimd.partition_all_reduce(cred[:], cneg[:], channels=128,
                                               reduce_op=bass_isa.ReduceOp.max)
                cs = acc.tile([1, 1], F32, name=f"cs_{it}")
                nc.vector.reduce_sum(cs[:], cred[0:1, :], axis=X)
                bt = acc.tile([1, 1], F32, name=f"bt_{it}")
                nc.vector.tensor_sub(bt[:], rsr[0:1, 0:1], cs[:])
                if it == 0:
                    nc.vector.tensor_copy(stot[:], bt[:])
                else:
                    nc.vector.tensor_add(stot[:], stot[:], bt[:])

            nc.scalar.mul(s_out[:], stot[:], 1.0 / (N * repeat))
            nc.gpsimd.dma_start(out_d[:], s_out[:])
    nc.compile()
    return nc


def _build_operands(x, y):
    """x,y [B,N,3] f32 -> per-core input maps (coordinate layouts)."""
    x = np.ascontiguousarray(x, np.float32)
    y = np.ascontiguousarray(y, np.float32)
    in_maps = []
    for core in range(NCORES):
        bs = range(core * BPC, (core + 1) * BPC)
        ys = np.concatenate([y[b].T for b in bs], axis=0)          # [BPC*3, N]
        xs_parts = []
        for b in bs:
            xb = x[b].reshape(RB, 128, 3)                           # [r, p, k]
            xs_parts.append(np.transpose(xb, (1, 2, 0)).reshape(128, 3 * RB))
        xs = np.concatenate(xs_parts, axis=1)                       # [128, BPC*3*RB]
        in_maps.append({"ys": np.ascontiguousarray(ys),
                        "xs": np.ascontiguousarray(xs)})
    return in_maps


_NC_CACHE = {}


def _get_nc(repeat: int = 1):
    if repeat not in _NC_CACHE:
        _NC_CACHE[repeat] = _build_nc(repeat)
    return _NC_CACHE[repeat]


def kernel(x, y):
    x = np.asarray(x, dtype=np.float32)
    y = np.asarray(y, dtype=np.float32)
    assert x.shape == (B, N, D3) and y.shape == (B, N, D3)
    in_maps = _build_operands(x, y)
    nc = _get_nc(1)
    res = run_bass_kernel_spmd(nc, in_maps, core_ids=list(range(NCORES)))
    total = sum(float(res.results[i]["out"][0, 0]) for i in range(NCORES))
    return np.float32(total / B)


# revision 9
# speedup vs baseline: 3.3864x; 1.0829x over previous
"""Chamfer distance kernel for Trainium2 (8 NeuronCores, data-parallel over batch).

Input : x, y float32 [16, 4096, 3]
Output: scalar float32 = mean_b [ mean_n min_m ||x_bn - y_bm||^2
                                + mean_m min_n ||x_bn - y_bm||^2 ]

Per core (2 batches). For each batch and each 128-row block of x points:
  s_k = Square(-yb_k + x_k)    (ScalarE; yb_k = y coord k broadcast to all
                                partitions [128,4096], x_k per-partition bias)
  d   = s_0 + s_1 + s_2        (VectorE adds)   -> d[p, m] = ||x_n - y_m||^2
  dirA: reduce_min(d) over m   -> row NN dist    (VectorE)
  dirB: colrun = min(colrun,d) running over row blocks (VectorE)
Batch finalize: colrun -> negate -> gpsimd partition_all_reduce(max) -> per-m
NN dists; reduce_sums + partition_all_reduce(add) -> scalar; accumulate.
Host: builds coordinate layouts (O(B*N)), shards batches 2-per-core, sums 8
partial sums / 16.
"""
import sys

sys.path.insert(0, "/opt/trn_rl_repo")

import numpy as np

import concourse.bacc as bacc
import concourse.bass as bass
import concourse.bass_isa as bass_isa
import concourse.tile as tile
from concourse import mybir
from concourse.alu_op_type import AluOpType
from concourse.bass_utils import run_bass_kernel_spmd

F32 = mybir.dt.float32
X = mybir.AxisListType.X
MIN = AluOpType.min
Square = mybir.ActivationFunctionType.Square

B, N, D3 = 16, 4096, 3
NCORES = 8
BPC = B // NCORES           # batches per core
RB = N // 128               # 32 row blocks


def _build_nc(repeat: int = 1):
    nc = bacc.Bacc("TRN2", target_bir_lowering=False, debug=False, num_devices=NCORES)
    # ys[b*4+k, m] = y[b, m, k] for k<3, y2[b, m] for k=3
    # xs[p, b*128 + k*32 + r] = 2*x[b, 128r+p, k] for k<3, x2[b, 128r+p] for k=3
    ys_d = nc.dram_tensor("ys", [BPC * 4, N], F32, kind="ExternalInput").ap()
    xs_d = nc.dram_tensor("xs", [128, BPC * 4 * RB], F32, kind="ExternalInput").ap()
    out_d = nc.dram_tensor("out", [1, 1], F32, kind="ExternalOutput").ap()

    with tile.TileContext(nc) as tc:
        import contextlib
        with contextlib.ExitStack() as ctx:
            const = ctx.enter_context(tc.tile_pool(name="const", bufs=1))
            acc = ctx.enter_context(tc.tile_pool(name="acc", bufs=1))
            ybp = ctx.enter_context(tc.tile_pool(name="ybp", bufs=4))
            wk = ctx.enter_context(tc.tile_pool(name="wk", bufs=1))

            xs_t = const.tile([128, BPC * 4 * RB], F32, name="xs_t")
            nc.gpsimd.dma_start(xs_t[:], xs_d[:])

            def load_yb(b):
                tiles = []
                for k in range(4):
                    t = ybp.tile([128, N], F32, name=f"yb{k}", tag="yb")
                    src = ys_d[4 * b + k : 4 * b + k + 1, :]
                    bcast = bass.AP(tensor=src.tensor, offset=src.offset,
                                    ap=[[0, 128]] + [list(p) for p in src.ap[1:]])
                    nc.gpsimd.dma_start(t[:], bcast)
                    tiles.append(t)
                return tiles

            colrun = acc.tile([128, N], F32, name="colrun")
            rowacc = acc.tile([128, RB], F32, name="rowacc")
            stot = acc.tile([1, 1], F32, name="stot")
            s_out = acc.tile([1, 1], F32, name="s_out")

            A = AluOpType
            for it in range(BPC * repeat):
                b = it % BPC
                yb = load_yb(b)

                def xsc(k, r):
                    o = b * 128 + k * RB + r
                    return xs_t[:, o : o + 1]

                for r in range(RB):
                    # u = 2x.y - x^2 - y^2 = -||x-y||^2, built with fused DVE ops
                    t0 = wk.tile([128, N], F32, name="t0", tag="t0")
                    nc.vector.tensor_scalar_mul(t0[:], yb[0][:], xsc(0, r))
                    a = wk.tile([128, N], F32, name="a", tag="a")
                    nc.vector.scalar_tensor_tensor(a[:], yb[1][:], xsc(1, r), t0[:],
                                                   op0=A.mult, op1=A.add)
                    c = wk.tile([128, N], F32, name="c", tag="c")
                    nc.vector.scalar_tensor_tensor(c[:], yb[2][:], xsc(2, r), a[:],
                                                   op0=A.mult, op1=A.add)
                    u = wk.tile([128, N], F32, name="u", tag="u")
                    nc.vector.scalar_tensor_tensor(u[:], c[:], xsc(3, r), yb[3][:],
                                                   op0=A.subtract, op1=A.subtract)
                    if r == 0:
                        nc.vector.tensor_copy(colrun[:], u[:])
                    else:
                        nc.vector.tensor_tensor(colrun[:], colrun[:], u[:], op=A.max)
                    nc.vector.tensor_reduce(rowacc[:, r : r + 1], u[:], axis=X, op=A.max)

                # batch finalize (all values are -min distances)
                rs = acc.tile([128, 1], F32, name=f"rs_{it}")
                nc.vector.reduce_sum(rs[:], rowacc[:], axis=X)
                rsr = acc.tile([128, 1], F32, name=f"rsr_{it}")
                nc.gpsimd.partition_all_reduce(rsr[:], rs[:], channels=128,
                                               reduce_op=bass_isa.ReduceOp.add)
                cred = wk.tile([128, N], F32, name="cred", tag="c")
                nc.gpsimd.partition_all_reduce(cred[:], colrun[:], channels=128,
                                               reduce_op=bass_isa.ReduceOp.max)
                cs = acc.tile([1, 1], F32, name=f"cs_{it}")
                nc.vector.reduce_sum(cs[:], cred[0:1, :], axis=X)
                bt = acc.tile([1, 1], F32, name=f"bt_{it}")
                nc.vector.tensor_add(bt[:], rsr[0:1, 0:1], cs[:])
                if it == 0:
                    nc.vector.tensor_copy(stot[:], bt[:])
                else:
                    nc.vector.tensor_add(stot[:], stot[:], bt[:])

            nc.scalar.mul(s_out[:], stot[:], -1.0 / (N * repeat))
            nc.gpsimd.dma_start(out_d[:], s_out[:])
    nc.compile()
    return nc


def _build_operands(x, y):
    """x,y [B,N,3] f32 -> per-core input maps (coordinate layouts)."""
    x = np.ascontiguousarray(x, np.float32)
    y = np.ascontiguousarray(y, np.float32)
    in_maps = []
    for core in range(NCORES):
        bs = range(core * BPC, (core + 1) * BPC)
        ys_parts, xs_parts = [], []
        for b in bs:
            y2 = (y[b] * y[b]).sum(axis=1, keepdims=True)           # [N,1]
            ys_parts.append(np.concatenate([y[b], y2], axis=1).T)   # [4, N]
            xb = x[b].reshape(RB, 128, 3)                           # [r, p, k]
            x2 = (x[b] * x[b]).sum(axis=1).reshape(RB, 128, 1)      # [r, p, 1]
            aug = np.concatenate([2.0 * xb, x2], axis=2)            # [r, p, 4]
            xs_parts.append(np.transpose(aug, (1, 2, 0)).reshape(128, 4 * RB))
        ys = np.concatenate(ys_parts, axis=0)                       # [BPC*4, N]
        xs = np.concatenate(xs_parts, axis=1)                       # [128, BPC*4*RB]
        in_maps.append({"ys": np.ascontiguousarray(ys),
                        "xs": np.ascontiguousarray(xs)})
    return in_maps


_NC_CACHE = {}


def _get_nc(repeat: int = 1):
    if repeat not in _NC_CACHE:
        _NC_CACHE[repeat] = _build_nc(repeat)
    return _NC_CACHE[repeat]


def kernel(x, y):
    x = np.asarray(x, dtype=np.float32)
    y = np.asarray(y, dtype=np.float32)
    assert x.shape == (B, N, D3) and y.shape == (B, N, D3)
    in_maps = _build_operands(x, y)
    nc = _get_nc(1)
    res = run_bass_kernel_spmd(nc, in_maps, core_ids=list(range(NCORES)))
    total = sum(float(res.results[i]["out"][0, 0]) for i in range(NCORES))
    return np.float32(total / B)


# revision 10
# speedup vs baseline: 4.4222x; 1.3059x over previous
"""Chamfer distance kernel for Trainium2 (8 NeuronCores, data-parallel over batch).

Input : x, y float32 [16, 4096, 3]
Output: scalar float32 = mean_b [ mean_n min_m ||x_bn - y_bm||^2
                                + mean_m min_n ||x_bn - y_bm||^2 ]

Per core (2 batches). For each batch and each 128-row block of x points:
  s_k = Square(-yb_k + x_k)    (ScalarE; yb_k = y coord k broadcast to all
                                partitions [128,4096], x_k per-partition bias)
  d   = s_0 + s_1 + s_2        (VectorE adds)   -> d[p, m] = ||x_n - y_m||^2
  dirA: reduce_min(d) over m   -> row NN dist    (VectorE)
  dirB: colrun = min(colrun,d) running over row blocks (VectorE)
Batch finalize: colrun -> negate -> gpsimd partition_all_reduce(max) -> per-m
NN dists; reduce_sums + partition_all_reduce(add) -> scalar; accumulate.
Host: builds coordinate layouts (O(B*N)), shards batches 2-per-core, sums 8
partial sums / 16.
"""
import sys

sys.path.insert(0, "/opt/trn_rl_repo")

import numpy as np

import concourse.bacc as bacc
import concourse.bass as bass
import concourse.bass_isa as bass_isa
import concourse.tile as tile
from concourse import mybir
from concourse.alu_op_type import AluOpType
from concourse.bass_utils import run_bass_kernel_spmd

F32 = mybir.dt.float32
X = mybir.AxisListType.X
MIN = AluOpType.min
Square = mybir.ActivationFunctionType.Square

B, N, D3 = 16, 4096, 3
NCORES = 8
BPC = B // NCORES           # batches per core
RB = N // 128               # 32 row blocks


def _build_nc(repeat: int = 1):
    nc = bacc.Bacc("TRN2", target_bir_lowering=False, debug=False, num_devices=NCORES)
    # ys[b*4+k, m] = y[b, m, k] for k<3, y2[b, m] for k=3
    # xs[p, b*128 + k*32 + r] = 2*x[b, 128r+p, k] for k<3, x2[b, 128r+p] for k=3
    ys_d = nc.dram_tensor("ys", [BPC * 4, N], F32, kind="ExternalInput").ap()
    xs_d = nc.dram_tensor("xs", [128, BPC * 4 * RB], F32, kind="ExternalInput").ap()
    out_d = nc.dram_tensor("out", [1, 1], F32, kind="ExternalOutput").ap()

    with tile.TileContext(nc) as tc:
        import contextlib
        with contextlib.ExitStack() as ctx:
            const = ctx.enter_context(tc.tile_pool(name="const", bufs=1))
            acc = ctx.enter_context(tc.tile_pool(name="acc", bufs=1))
            ybp = ctx.enter_context(tc.tile_pool(name="ybp", bufs=4))
            wk = ctx.enter_context(tc.tile_pool(name="wk", bufs=1))

            xs_t = const.tile([128, BPC * 4 * RB], F32, name="xs_t")
            nc.gpsimd.dma_start(xs_t[:], xs_d[:])

            def load_yb(b):
                tiles = []
                for k in range(4):
                    t = ybp.tile([128, N], F32, name=f"yb{k}", tag="yb")
                    src = ys_d[4 * b + k : 4 * b + k + 1, :]
                    bcast = bass.AP(tensor=src.tensor, offset=src.offset,
                                    ap=[[0, 128]] + [list(p) for p in src.ap[1:]])
                    nc.gpsimd.dma_start(t[:], bcast)
                    tiles.append(t)
                return tiles

            colrun = acc.tile([128, N], F32, name="colrun")
            rowacc = acc.tile([128, RB], F32, name="rowacc")
            stot = acc.tile([1, 1], F32, name="stot")
            s_out = acc.tile([1, 1], F32, name="s_out")
            # fixed work tiles: VectorE is serial, so plain WAW reuse is free
            t0 = wk.tile([128, N], F32, name="t0")
            a = wk.tile([128, N], F32, name="a")
            c = wk.tile([128, N], F32, name="c")
            u = wk.tile([128, N], F32, name="u")
            cred = wk.tile([128, N], F32, name="cred")

            A = AluOpType
            for it in range(BPC * repeat):
                b = it % BPC
                yb = load_yb(b)

                def xsc(k, r):
                    o = b * 128 + k * RB + r
                    return xs_t[:, o : o + 1]

                for r in range(RB):
                    # u = 2x.y - x^2 - y^2 = -||x-y||^2, built with fused DVE ops
                    nc.vector.tensor_scalar_mul(t0[:], yb[0][:], xsc(0, r))
                    nc.vector.scalar_tensor_tensor(a[:], yb[1][:], xsc(1, r), t0[:],
                                                   op0=A.mult, op1=A.add)
                    nc.vector.scalar_tensor_tensor(c[:], yb[2][:], xsc(2, r), a[:],
                                                   op0=A.mult, op1=A.add)
                    nc.vector.scalar_tensor_tensor(u[:], c[:], xsc(3, r), yb[3][:],
                                                   op0=A.subtract, op1=A.subtract)
                    if r == 0:
                        nc.vector.tensor_copy(colrun[:], u[:])
                    else:
                        nc.vector.tensor_tensor(colrun[:], colrun[:], u[:], op=A.max)
                    nc.vector.tensor_reduce(rowacc[:, r : r + 1], u[:], axis=X, op=A.max)

                # batch finalize (all values are -min distances)
                rs = acc.tile([128, 1], F32, name=f"rs_{it}")
                nc.vector.reduce_sum(rs[:], rowacc[:], axis=X)
                rsr = acc.tile([128, 1], F32, name=f"rsr_{it}")
                nc.gpsimd.partition_all_reduce(rsr[:], rs[:], channels=128,
                                               reduce_op=bass_isa.ReduceOp.add)
                nc.gpsimd.partition_all_reduce(cred[:], colrun[:], channels=128,
                                               reduce_op=bass_isa.ReduceOp.max)
                cs = acc.tile([1, 1], F32, name=f"cs_{it}")
                nc.vector.reduce_sum(cs[:], cred[0:1, :], axis=X)
                bt = acc.tile([1, 1], F32, name=f"bt_{it}")
                nc.vector.tensor_add(bt[:], rsr[0:1, 0:1], cs[:])
                if it == 0:
                    nc.vector.tensor_copy(stot[:], bt[:])
                else:
                    nc.vector.tensor_add(stot[:], stot[:], bt[:])

            nc.scalar.mul(s_out[:], stot[:], -1.0 / (N * repeat))
            nc.gpsimd.dma_start(out_d[:], s_out[:])
    nc.compile()
    return nc


def _build_operands(x, y):
    """x,y [B,N,3] f32 -> per-core input maps (coordinate layouts)."""
    x = np.ascontiguousarray(x, np.float32)
    y = np.ascontiguousarray(y, np.float32)
    in_maps = []
    for core in range(NCORES):
        bs = range(core * BPC, (core + 1) * BPC)
        ys_parts, xs_parts = [], []
        for b in bs:
            y2 = (y[b] * y[b]).sum(axis=1, keepdims=True)           # [N,1]
            ys_parts.append(np.concatenate([y[b], y2], axis=1).T)   # [4, N]
            xb = x[b].reshape(RB, 128, 3)                           # [r, p, k]
            x2 = (x[b] * x[b]).sum(axis=1).reshape(RB, 128, 1)      # [r, p, 1]
            aug = np.concatenate([2.0 * xb, x2], axis=2)            # [r, p, 4]
            xs_parts.append(np.transpose(aug, (1, 2, 0)).reshape(128, 4 * RB))
        ys = np.concatenate(ys_parts, axis=0)                       # [BPC*4, N]
        xs = np.concatenate(xs_parts, axis=1)                       # [128, BPC*4*RB]
        in_maps.append({"ys": np.ascontiguousarray(ys),
                        "xs": np.ascontiguousarray(xs)})
    return in_maps


_NC_CACHE = {}


def _get_nc(repeat: int = 1):
    if repeat not in _NC_CACHE:
        _NC_CACHE[repeat] = _build_nc(repeat)
    return _NC_CACHE[repeat]


def kernel(x, y):
    x = np.asarray(x, dtype=np.float32)
    y = np.asarray(y, dtype=np.float32)
    assert x.shape == (B, N, D3) and y.shape == (B, N, D3)
    in_maps = _build_operands(x, y)
    nc = _get_nc(1)
    res = run_bass_kernel_spmd(nc, in_maps, core_ids=list(range(NCORES)))
    total = sum(float(res.results[i]["out"][0, 0]) for i in range(NCORES))
    return np.float32(total / B)


# revision 11
# speedup vs baseline: 4.4859x; 1.0144x over previous
"""Chamfer distance kernel for Trainium2 (8 NeuronCores, data-parallel over batch).

Input : x, y float32 [16, 4096, 3]
Output: scalar float32 = mean_b [ mean_n min_m ||x_bn - y_bm||^2
                                + mean_m min_n ||x_bn - y_bm||^2 ]

Per core (2 batches). For each batch and each 128-row block of x points:
  s_k = Square(-yb_k + x_k)    (ScalarE; yb_k = y coord k broadcast to all
                                partitions [128,4096], x_k per-partition bias)
  d   = s_0 + s_1 + s_2        (VectorE adds)   -> d[p, m] = ||x_n - y_m||^2
  dirA: reduce_min(d) over m   -> row NN dist    (VectorE)
  dirB: colrun = min(colrun,d) running over row blocks (VectorE)
Batch finalize: colrun -> negate -> gpsimd partition_all_reduce(max) -> per-m
NN dists; reduce_sums + partition_all_reduce(add) -> scalar; accumulate.
Host: builds coordinate layouts (O(B*N)), shards batches 2-per-core, sums 8
partial sums / 16.
"""
import sys

sys.path.insert(0, "/opt/trn_rl_repo")

import numpy as np

import concourse.bacc as bacc
import concourse.bass as bass
import concourse.bass_isa as bass_isa
import concourse.tile as tile
from concourse import mybir
from concourse.alu_op_type import AluOpType
from concourse.bass_utils import run_bass_kernel_spmd

F32 = mybir.dt.float32
X = mybir.AxisListType.X
MIN = AluOpType.min
Square = mybir.ActivationFunctionType.Square

B, N, D3 = 16, 4096, 3
NCORES = 8
BPC = B // NCORES           # batches per core
RB = N // 128               # 32 row blocks


def _build_nc(repeat: int = 1):
    nc = bacc.Bacc("TRN2", target_bir_lowering=False, debug=False, num_devices=NCORES)
    # ys[b*4+k, m] = y[b, m, k] for k<3, y2[b, m] for k=3
    # xs[p, b*128 + k*32 + r] = 2*x[b, 128r+p, k] for k<3, x2[b, 128r+p] for k=3
    ys_d = nc.dram_tensor("ys", [BPC * 4, N], F32, kind="ExternalInput").ap()
    xs_d = nc.dram_tensor("xs", [128, BPC * 4 * RB], F32, kind="ExternalInput").ap()
    out_d = nc.dram_tensor("out", [1, 1], F32, kind="ExternalOutput").ap()

    with tile.TileContext(nc) as tc:
        import contextlib
        with contextlib.ExitStack() as ctx:
            const = ctx.enter_context(tc.tile_pool(name="const", bufs=1))
            acc = ctx.enter_context(tc.tile_pool(name="acc", bufs=1))
            ybp = ctx.enter_context(tc.tile_pool(name="ybp", bufs=4))
            wk = ctx.enter_context(tc.tile_pool(name="wk", bufs=1))

            xs_t = const.tile([128, BPC * 4 * RB], F32, name="xs_t")
            nc.gpsimd.dma_start(xs_t[:], xs_d[:])

            def load_yb(b):
                tiles = []
                for k in range(4):
                    t = ybp.tile([128, N], F32, name=f"yb{k}", tag="yb")
                    src = ys_d[4 * b + k : 4 * b + k + 1, :]
                    bcast = bass.AP(tensor=src.tensor, offset=src.offset,
                                    ap=[[0, 128]] + [list(p) for p in src.ap[1:]])
                    nc.gpsimd.dma_start(t[:], bcast)
                    tiles.append(t)
                return tiles

            colrun = acc.tile([128, N], F32, name="colrun")
            rowacc = acc.tile([128, RB], F32, name="rowacc")
            stot = acc.tile([1, 1], F32, name="stot")
            s_out = acc.tile([1, 1], F32, name="s_out")
            # fixed work tiles: VectorE is serial, so plain WAW reuse is free
            t0 = wk.tile([128, N], F32, name="t0")
            a = wk.tile([128, N], F32, name="a")
            c = wk.tile([128, N], F32, name="c")
            u2 = wk.tile([128, 2 * N], F32, name="u2")
            cred = wk.tile([128, N], F32, name="cred")

            A = AluOpType
            for it in range(BPC * repeat):
                b = it % BPC
                yb = load_yb(b)

                def xsc(k, r):
                    o = b * 128 + k * RB + r
                    return xs_t[:, o : o + 1]

                for r in range(0, RB, 2):
                    # u = 2x.y - x^2 - y^2 = -||x-y||^2; two row blocks per pass
                    for h in (0, 1):
                        rr = r + h
                        seg = u2[:, h * N : (h + 1) * N]
                        nc.vector.tensor_scalar_mul(t0[:], yb[0][:], xsc(0, rr))
                        nc.vector.scalar_tensor_tensor(a[:], yb[1][:], xsc(1, rr), t0[:],
                                                       op0=A.mult, op1=A.add)
                        nc.vector.scalar_tensor_tensor(c[:], yb[2][:], xsc(2, rr), a[:],
                                                       op0=A.mult, op1=A.add)
                        nc.vector.scalar_tensor_tensor(seg, c[:], xsc(3, rr), yb[3][:],
                                                       op0=A.subtract, op1=A.subtract)
                    # fold the pair, update running column max, and one 2-wide row reduce
                    nc.vector.tensor_tensor(t0[:], u2[:, 0:N], u2[:, N : 2 * N], op=A.max)
                    if r == 0:
                        nc.vector.tensor_copy(colrun[:], t0[:])
                    else:
                        nc.vector.tensor_tensor(colrun[:], colrun[:], t0[:], op=A.max)
                    nc.vector.tensor_reduce(rowacc[:, r : r + 2],
                                            u2[:].rearrange("p (h n) -> p h n", h=2),
                                            axis=X, op=A.max)

                # batch finalize (all values are -min distances)
                rs = acc.tile([128, 1], F32, name=f"rs_{it}")
                nc.vector.reduce_sum(rs[:], rowacc[:], axis=X)
                rsr = acc.tile([128, 1], F32, name=f"rsr_{it}")
                nc.gpsimd.partition_all_reduce(rsr[:], rs[:], channels=128,
                                               reduce_op=bass_isa.ReduceOp.add)
                nc.gpsimd.partition_all_reduce(cred[:], colrun[:], channels=128,
                                               reduce_op=bass_isa.ReduceOp.max)
                cs = acc.tile([1, 1], F32, name=f"cs_{it}")
                nc.vector.reduce_sum(cs[:], cred[0:1, :], axis=X)
                bt = acc.tile([1, 1], F32, name=f"bt_{it}")
                nc.vector.tensor_add(bt[:], rsr[0:1, 0:1], cs[:])
                if it == 0:
                    nc.vector.tensor_copy(stot[:], bt[:])
                else:
                    nc.vector.tensor_add(stot[:], stot[:], bt[:])

            nc.scalar.mul(s_out[:], stot[:], -1.0 / (N * repeat))
            nc.gpsimd.dma_start(out_d[:], s_out[:])
    nc.compile()
    return nc


def _build_operands(x, y):
    """x,y [B,N,3] f32 -> per-core input maps (coordinate layouts)."""
    x = np.ascontiguousarray(x, np.float32)
    y = np.ascontiguousarray(y, np.float32)
    in_maps = []
    for core in range(NCORES):
        bs = range(core * BPC, (core + 1) * BPC)
        ys_parts, xs_parts = [], []
        for b in bs:
            y2 = (y[b] * y[b]).sum(axis=1, keepdims=True)           # [N,1]
            ys_parts.append(np.concatenate([y[b], y2], axis=1).T)   # [4, N]
            xb = x[b].reshape(RB, 128, 3)                           # [r, p, k]
            x2 = (x[b] * x[b]).sum(axis=1).reshape(RB, 128, 1)      # [r, p, 1]
            aug = np.concatenate([2.0 * xb, x2], axis=2)            # [r, p, 4]
            xs_parts.append(np.transpose(aug, (1, 2, 0)).reshape(128, 4 * RB))
        ys = np.concatenate(ys_parts, axis=0)                       # [BPC*4, N]
        xs = np.concatenate(xs_parts, axis=1)                       # [128, BPC*4*RB]
        in_maps.append({"ys": np.ascontiguousarray(ys),
                        "xs": np.ascontiguousarray(xs)})
    return in_maps


_NC_CACHE = {}


def _get_nc(repeat: int = 1):
    if repeat not in _NC_CACHE:
        _NC_CACHE[repeat] = _build_nc(repeat)
    return _NC_CACHE[repeat]


def kernel(x, y):
    x = np.asarray(x, dtype=np.float32)
    y = np.asarray(y, dtype=np.float32)
    assert x.shape == (B, N, D3) and y.shape == (B, N, D3)
    in_maps = _build_operands(x, y)
    nc = _get_nc(1)
    res = run_bass_kernel_spmd(nc, in_maps, core_ids=list(range(NCORES)))
    total = sum(float(res.results[i]["out"][0, 0]) for i in range(NCORES))
    return np.float32(total / B)
